# revision 80
# baseline (speedup 1.0000x reference)
"""Causal self-attention (B=4, T=2048, D=1024, H=16) on 8 TRN2 NeuronCores.

Sharding: core c handles batch b = c // 2 and head-group g = c % 2
(8 heads = 512 of the 1024 feature dims). Each core:
  1. QKV projection for its head-group's columns. q, k are produced
     TRANSPOSED ([feat, tok], feature dim on partitions) so they feed the
     attention matmuls directly; v is produced natural ([tok, feat]) so it
     is the PV stationary operand.
  2. RoPE via a PE rotation matmul (rotate_half as a constant 128x128
     block-diagonal permutation) + DVE combine with cos/sin.
  3. Causal attention with scores in [k, q] orientation: exp(score/8 - 2)
     without max-subtraction (shift-invariant), row-sum obtained free via a
     ones-column appended to v (PV matmul M=65: rows 0-63 = y, row 64 =
     softmax denominator).
  4. Late softmax normalization (reciprocal + gpsimd partition-broadcast),
     then the partial output projection with its 512 rows of W_out.
Host sums the two head-group partials per batch and adds b_out.

Schedule: token chunks (512 q each) outermost, head pairs inner; each
block's thin diagonal score tiles run first so their longer
scores->exp->PV chains overlap the dense tiles' PE work. Projection /
v / out-proj work items stream into the attention pipeline's PE gaps
under a per-chunk quota that saves the out-projections for the last
(exp-bound) chunk; the last chunk's out-proj splits its accumulation so
only the final head-pair matmuls wait on the last epilogue. DMA layouts
keep per-partition runs >= 512B (sub-512B runs pay 2x in the DMA
engines) and the startup loads alternate the two HWDGE queues in
consumption order.

All matmuls run in bf16 (fp32 matmul is 1/4 rate on the PE; fp8 PV
fails the accuracy gate); softmax statistics accumulate in fp32 PSUM.
"""

import numpy as np
import ml_dtypes

import concourse.tile as tile
from concourse import bacc, mybir
from concourse.bass_utils import run_bass_kernel_spmd

dt = mybir.dt
bf16 = ml_dtypes.bfloat16

B, T, C = 4, 2048, 1024
H, HD = 16, 64
N_CORES = 8
HPC = 8          # heads per core
FPC = H // 2 * HD // 8 * 8 // 2 * 2  # = 512 features per core (q, k, v each)
KSUB = C // 128  # 8 contraction subtiles
TT = T // 128    # 16 token tiles
TC = T // 512    # 4 token chunks

_compiled = None


def _build():
    nc = bacc.Bacc()
    dts = dt.bfloat16

    xT = nc.dram_tensor("xT", [TC, 128, KSUB, 512], dts, kind="ExternalInput")
    Wqk = nc.dram_tensor("Wqk", [8, 128, KSUB, 128], dts, kind="ExternalInput")
    Wv = nc.dram_tensor("Wv", [128, KSUB, 512], dts, kind="ExternalInput")
    Wo = nc.dram_tensor("Wo", [128, 4, 1024], dts, kind="ExternalInput")
    bqk = nc.dram_tensor("bqk", [128, 8], dt.float32, kind="ExternalInput")
    bv = nc.dram_tensor("bv", [128, 512], dt.float32, kind="ExternalInput")
    RT = nc.dram_tensor("RT", [128, 128], dts, kind="ExternalInput")
    cosd = nc.dram_tensor("cosd", [TC, 128, 512], dts, kind="ExternalInput")
    sind = nc.dram_tensor("sind", [TC, 128, 512], dts, kind="ExternalInput")
    maskd = nc.dram_tensor("maskd", [128, 128], dts, kind="ExternalInput")
    out = nc.dram_tensor("out", [T, C], dts, kind="ExternalOutput")

    with tile.TileContext(nc) as tc:
        with (
            tc.tile_pool(name="weights", bufs=1) as wp,
            tc.tile_pool(name="acts", bufs=1) as ap,
            tc.tile_pool(name="scratch", bufs=3) as sp,
            tc.tile_pool(name="exps", bufs=6) as ep,
            tc.tile_pool(name="norm", bufs=2) as np_,
            tc.tile_pool(name="outs", bufs=6) as op,
            tc.tile_pool(name="psum", bufs=2, space="PSUM") as pp,
            tc.tile_pool(name="psum_big", bufs=2, space="PSUM") as pb,
            tc.tile_pool(name="psum_pv", bufs=2, space="PSUM") as ppv,
        ):
            # chunk-major xT and fs-major Wqk: DMA destination runs are
            # 8KB/2KB contiguous per partition (sub-512B runs pay 2x in the
            # DMA engines)
            xT_sb = wp.tile([128, TC, KSUB, 512], dts)
            Wqk_sb = wp.tile([128, 8, KSUB, 128], dts)
            Wv_sb = wp.tile([128, KSUB, 512], dts)
            Wo_sb = wp.tile([128, 4, 1024], dts)
            bqk_sb = wp.tile([128, 8], dt.float32)
            bv_sb = wp.tile([128, 512], dt.float32)
            RT_sb = wp.tile([128, 128], dts)
            cos_sb = wp.tile([128, T], dts)
            sin_sb = wp.tile([128, T], dts)
            mask_sb = wp.tile([128, 128], dts)
            # exp bias constant (-2): keeps fp8e4m3 exp outputs under the
            # 448 max (softmax is shift-invariant, numerator and denominator
            # both scale by e^-2)
            nb2 = wp.tile([128, 1], dt.float32)
            nc.vector.memset(nb2[:], -2.0)

            def load_xT(c4):
                nc.sync.dma_start(xT_sb[:, c4], xT[c4])

            def load_wqk(fs):
                nc.sync.dma_start(Wqk_sb[:, fs], Wqk[fs])

            # first-needed data up front, in consumption order, alternating
            # the two HWDGE queues (SP + Activation) so descriptor issue
            # (fixed cost per DMA) pipelines with the transfers
            cosv = cos_sb.rearrange("p (c q) -> p c q", c=TC)
            sinv = sin_sb.rearrange("p (c q) -> p c q", c=TC)
            load_wqk(0)
            nc.scalar.dma_start(xT_sb[:, 0, 0:4], xT[0][:, 0:4, :])
            nc.scalar.dma_start(Wqk_sb[:, 4], Wqk[4])
            nc.sync.dma_start(xT_sb[:, 0, 4:8], xT[0][:, 4:8, :])
            nc.scalar.dma_start(bqk_sb[:], bqk[:])
            nc.sync.dma_start(cosv[:, 0], cosd[0])
            nc.scalar.dma_start(sinv[:, 0], sind[0])
            nc.sync.dma_start(RT_sb[:], RT[:])
            nc.sync.dma_start(mask_sb[:], maskd[:])
            load_wqk(1)
            nc.scalar.dma_start(Wqk_sb[:, 5], Wqk[5])
            load_wqk(2)
            nc.scalar.dma_start(Wqk_sb[:, 6], Wqk[6])
            load_wqk(3)
            nc.scalar.dma_start(Wqk_sb[:, 7], Wqk[7])
            nc.sync.dma_start(Wv_sb[:], Wv[:])
            nc.scalar.dma_start(bv_sb[:], bv[:])
            load_xT(1)
            nc.scalar.dma_start(cosv[:, 1], cosd[1])
            nc.scalar.dma_start(sinv[:, 1], sind[1])
            nc.sync.dma_start(Wo_sb[:], Wo[:])
            for c4 in range(2, TC):
                load_xT(c4)
                nc.sync.dma_start(cosv[:, c4], cosd[c4])
                nc.sync.dma_start(sinv[:, c4], sind[c4])

            qT_sb = ap.tile([128, 4, T], dts)   # rope'd q, [feat, tok]
            kT_sb = ap.tile([128, 4, T], dts)   # rope'd k, [feat, tok]
            # v natural + ones col per head (bf16: fp8 v fails the accuracy
            # gate — concentrated attention passes v's quantization error
            # straight through)
            v_sb = ap.tile([128, TT, 8 * 65], dts)
            # normalized attention out (out-proj lhsT), one tile per token
            # chunk so out-proj of chunk c has no (tracker-level) dependency
            # on later chunks' y writes
            y_tiles = [ap.tile([128, 4, 512], dts, name=f"y{c}")
                       for c in range(TC)]

            # ones columns of v (col 64 of each head's 65-wide block)
            v_heads = v_sb.rearrange("p t (h f) -> p t h f", h=8)
            nc.vector.memset(v_heads[:, :, :, 64], 1.0)

            # ---- fine-grained work emitters -----------------------------
            def v_tile(tt):
                psv = pp.tile([128, 512], dt.float32, tag="ps512")
                for ks in range(KSUB):
                    nc.tensor.matmul(
                        psv[:],
                        xT_sb[:, tt // 4, ks, (tt % 4) * 128:(tt % 4 + 1) * 128],
                        Wv_sb[:, ks, :],
                        start=(ks == 0), stop=(ks == KSUB - 1),
                    )
                nc.vector.tensor_add(
                    v_heads[:, tt, :, 0:64],
                    psv[:].rearrange("p (h f) -> p h f", h=8),
                    bv_sb[:].rearrange("p (h f) -> p h f", h=8),
                )

            def proj_rope(fs, c4, use_big=False):
                tsl = slice(c4 * 512, (c4 + 1) * 512)
                if use_big:
                    # bootstrap: attention pools are idle, borrow a big tile
                    bigt = pb.tile([128, 1024], dt.float32, tag="big")
                    ps, rps = bigt[:, 0:512], bigt[:, 512:1024]
                else:
                    # single tile: the rope matmul reuses ps once the bias
                    # extract has read it, so two projs pipeline through the
                    # two pp buffers instead of one
                    ps = pp.tile([128, 512], dt.float32, tag="ps512")
                    rps = ps
                for ks in range(KSUB):
                    nc.tensor.matmul(
                        ps[:],
                        Wqk_sb[:, fs, ks, :],
                        xT_sb[:, c4, ks, :],
                        start=(ks == 0), stop=(ks == KSUB - 1),
                    )
                qb = sp.tile([128, 512], dt.float32, tag="qb")
                nc.vector.tensor_scalar_add(qb[:], ps[:], bqk_sb[:, fs:fs + 1])
                u = sp.tile([128, 512], dts, tag="u")
                nc.vector.tensor_mul(u[:], qb[:], sin_sb[:, tsl])
                w = sp.tile([128, 512], dt.float32, tag="w")
                nc.vector.tensor_mul(w[:], qb[:], cos_sb[:, tsl])
                nc.tensor.matmul(rps[:], RT_sb[:], u[:], start=True, stop=True)
                dst = qT_sb if fs < 4 else kT_sb
                nc.vector.tensor_add(dst[:, fs % 4, tsl], w[:], rps[:])

            def out_proj(tt):
                yt = y_tiles[tt // 4]
                t0 = (tt % 4) * 128
                for n2 in range(2):
                    po = pp.tile([128, 512], dt.float32, tag="ps512")
                    for s in range(4):
                        nc.tensor.matmul(
                            po[:],
                            yt[:, s, t0:t0 + 128],
                            Wo_sb[:, s, n2 * 512:(n2 + 1) * 512],
                            start=(s == 0), stop=(s == 3),
                        )
                    ost = op.tile([128, 512], dts, tag="ost")
                    nc.vector.tensor_copy(ost[:], po[:])
                    nc.sync.dma_start(
                        out[tt * 128:(tt + 1) * 128, n2 * 512:(n2 + 1) * 512],
                        ost[:],
                    )

            from collections import deque

            # work items streamed into the attention pipeline's PE gaps.
            # Projections/v for chunk c must finish before chunk c's
            # attention; out-proj items are appended once a chunk's y is
            # final and are held back preferentially for the (long) last
            # chunk's j-loop, which otherwise runs dry and stalls on exp.
            fillers = deque()
            for c4 in range(1, TC):
                for fs in (0, 4, 1, 5, 2, 6, 3, 7):
                    fillers.append(("proj", fs, c4))
                for tt in range(4 * c4, 4 * c4 + 4):
                    fillers.append(("v", tt))

            def out_tail(tt):
                # last chunk's out-proj: accumulate the first three
                # head-pair contributions in a wide pb tile (hoistable ahead
                # of the final epilogue); only the s=3 matmuls and the copy
                # wait on the last y write
                yt = y_tiles[TC - 1]
                t0 = (tt % 4) * 128
                po2 = pb.tile([128, 1024], dt.float32, tag="big")
                for n2 in range(2):
                    for s in range(3):
                        nc.tensor.matmul(
                            po2[:, n2 * 512:(n2 + 1) * 512],
                            yt[:, s, t0:t0 + 128],
                            Wo_sb[:, s, n2 * 512:(n2 + 1) * 512],
                            start=(s == 0), stop=False,
                            skip_group_check=True,
                        )
                ost2 = op.tile([128, 1024], dts, tag="ost2")
                for n2 in range(2):
                    nc.tensor.matmul(
                        po2[:, n2 * 512:(n2 + 1) * 512],
                        yt[:, 3, t0:t0 + 128],
                        Wo_sb[:, 3, n2 * 512:(n2 + 1) * 512],
                        start=False, stop=True,
                        skip_group_check=True,
                    )
                    # copy each half right after its s=3 matmul so the copy
                    # train pipelines with the remaining tail matmuls
                    nc.vector.tensor_copy(
                        ost2[:, n2 * 512:(n2 + 1) * 512],
                        po2[:, n2 * 512:(n2 + 1) * 512])
                    deng = nc.scalar if (tt + n2) % 2 else nc.sync
                    deng.dma_start(
                        out[tt * 128:(tt + 1) * 128,
                            n2 * 512:(n2 + 1) * 512],
                        ost2[:, n2 * 512:(n2 + 1) * 512])

            def run_item(it):
                if it[0] == "v":
                    v_tile(it[1])
                elif it[0] == "proj":
                    proj_rope(it[1], it[2])
                elif it[0] == "outt":
                    out_tail(it[1])
                else:
                    out_proj(it[1])

            def drain_needed(qc):
                rest = deque()
                while fillers:
                    it = fillers.popleft()
                    if (it[0] == "proj" and it[2] <= qc) or \
                       (it[0] == "v" and it[1] < 4 * qc + 4):
                        run_item(it)
                    else:
                        rest.append(it)
                fillers.extend(rest)

            # bootstrap: all head pairs' q/k for chunk 0 + v for chunk 0
            for fs in (0, 4, 1, 5, 2, 6, 3, 7):
                proj_rope(fs, 0, use_big=True)
            for tt in range(4):
                v_tile(tt)

            for qc in range(TC):
                if qc:
                    drain_needed(qc)
                iters = 4 * (4 * qc + 4)
                # quota: earlier chunks emit only the next chunk's deps
                # (their attention windows are PE-bound); all out-proj work
                # is held for the last chunk's ACT-bound window, minus two
                # items kept past the loop to cover the final epilogue
                if qc == TC - 1:
                    quota = max(0, len(fillers) - 2)
                else:
                    quota = min(len(fillers), 12)
                pops = it_count = 0
                for hp in range(4):
                    qsl = slice(qc * 512, (qc + 1) * 512)
                    jmax = 4 * qc + 3
                    pv0 = ppv.tile([65, 512], dt.float32, tag="pv")
                    pv1 = ppv.tile([65, 512], dt.float32, tag="pv")
                    # diagonal (thin) tiles first: their longer dependency
                    # chains overlap the dense tiles' PE work instead of
                    # piling up at the block end
                    js = list(range(4 * qc, 4 * qc + 4)) + list(range(4 * qc))
                    for ji, j in enumerate(js):
                        qs = max(0, j * 128 - qc * 512)
                        diag = j >= 4 * qc
                        big = pb.tile([128, 1024], dt.float32, tag="big")
                        for par in range(2):
                            kb = par * 64
                            o = par * 512
                            nc.tensor.matmul(
                                big[:, o + qs:o + 512],
                                kT_sb[kb:kb + 64, hp, j * 128:(j + 1) * 128],
                                qT_sb[kb:kb + 64, hp,
                                      qc * 512 + qs:(qc + 1) * 512],
                                start=True, stop=True,
                            )
                        big_v = big.rearrange("p (two q) -> p two q", two=2)
                        ex = ep.tile([128, 1024], dts, tag="ex")
                        ex_v = ex.rearrange("p (two q) -> p two q", two=2)
                        nc.scalar.activation(
                            ex_v[:, :, qs:512], big_v[:, :, qs:512],
                            mybir.ActivationFunctionType.Exp,
                            bias=nb2[:], scale=0.125,
                        )
                        if diag:
                            # diagonal tile: zero the strictly-upper part
                            nc.vector.tensor_tensor(
                                ex_v[:, :, qs:qs + 128],
                                ex_v[:, :, qs:qs + 128],
                                mask_sb[:, None, :].to_broadcast((128, 2, 128)),
                                mybir.AluOpType.mult,
                            )
                        for par in range(2):
                            h = 2 * hp + par
                            pv = pv0 if par == 0 else pv1
                            nc.tensor.matmul(
                                pv[:, qs:512],
                                v_sb[:, j, h * 65:(h + 1) * 65],
                                ex[:, par * 512 + qs:par * 512 + 512],
                                start=(ji == 0), stop=(ji == jmax),
                            )
                        it_count += 1
                        while (ji < jmax and fillers and pops < quota
                               and pops * iters < quota * it_count):
                            run_item(fillers.popleft())
                            pops += 1
                    last = qc == TC - 1 and hp == 3
                    for par in range(2):
                        h = 2 * hp + par
                        kb = par * 64
                        pv = pv0 if par == 0 else pv1
                        if not last and qc >= 2:
                            # free the pv PSUM bank fast (the next block's
                            # first PV reuses it): evacuate to SBUF, then
                            # normalize from the copy
                            pvc = np_.tile([65, 512], dt.float32, tag="pvc")
                            nc.vector.tensor_copy(pvc[:], pv[:])
                            pv = pvc
                        rinv = np_.tile([1, 512], dt.float32, tag="rinv")
                        nc.vector.reciprocal(rinv[0:1, :], pv[64:65, :])
                        rb = np_.tile([64, 512], dt.float32, tag="rb")
                        nc.gpsimd.partition_broadcast(rb[:], rinv[0:1, :])
                        nc.vector.tensor_mul(
                            y_tiles[qc][kb:kb + 64, hp, :], pv[0:64, :], rb[:],
                        )
                    while (fillers and pops < quota
                           and pops * iters < quota * it_count):
                        run_item(fillers.popleft())
                        pops += 1
                    if hp == 3:
                        # this chunk's y is final for all heads: stream out-proj
                        kind = "outt" if qc == TC - 1 else "out"
                        for tt in range(4 * qc, 4 * qc + 4):
                            fillers.append((kind, tt))
            while fillers:
                run_item(fillers.popleft())

    nc.compile()
    return nc


def _prep_core_inputs(x, W_qkv, b_qkv, W_out, g):
    """Host-side shard prep for head-group g (features g*512:(g+1)*512)."""
    fs = slice(g * 512, (g + 1) * 512)
    Wq = W_qkv[:, 0:1024][:, fs]          # [1024, 512]
    Wk = W_qkv[:, 1024:2048][:, fs]
    Wv_ = W_qkv[:, 2048:3072][:, fs]
    bq = b_qkv[0:1024][fs]
    bk = b_qkv[1024:2048][fs]
    bv_ = b_qkv[2048:3072][fs]

    Wqk_np = np.concatenate([Wq, Wk], axis=1)        # [1024, 1024]
    # [fs, p, ks, col]
    Wqk_np = Wqk_np.reshape(KSUB, 128, 8, 128).transpose(2, 1, 0, 3)
    Wv_np = Wv_.reshape(KSUB, 128, 512).transpose(1, 0, 2)
    Wo_np = W_out[fs, :].reshape(4, 128, 1024).transpose(1, 0, 2)
    bqk_np = np.concatenate([bq, bk]).reshape(8, 128).T.copy()   # [128, 8]
    bv_np = np.broadcast_to(bv_[None, :], (128, 512)).copy()

    return {
        "Wqk": np.ascontiguousarray(Wqk_np).astype(bf16),
        "Wv": np.ascontiguousarray(Wv_np).astype(bf16),
        "Wo": np.ascontiguousarray(Wo_np).astype(bf16),
        "bqk": np.ascontiguousarray(bqk_np).astype(np.float32),
        "bv": bv_np.astype(np.float32),
    }


def _shared_inputs():
    # rotation matrix: (R q)[d] = -q[d+32] for d<32, q[d-32] for 32<=d<64
    R64 = np.zeros((64, 64), dtype=np.float32)
    for d in range(32):
        R64[d, d + 32] = -1.0
        R64[d + 32, d] = 1.0
    R128 = np.zeros((128, 128), dtype=np.float32)
    R128[0:64, 0:64] = R64
    R128[64:128, 64:128] = R64
    RT_np = R128.T.copy()

    inv_freq = 1.0 / (10000.0 ** (np.arange(0, HD, 2, dtype=np.float32) / HD))
    t = np.arange(T, dtype=np.float32)
    freqs = np.outer(t, inv_freq)                     # [T, 32]
    p = np.arange(128)
    cos_np = np.cos(freqs[:, p % 32]).T.copy()        # [128, T]
    sin_np = np.sin(freqs[:, p % 32]).T.copy()

    # causal mask for the diagonal 128-block: mask[k, q] = 1 iff k <= q
    mask_np = np.tril(np.ones((128, 128), dtype=np.float32)).T.copy()

    return {
        "RT": RT_np.astype(bf16),
        "cosd": np.ascontiguousarray(
            cos_np.reshape(128, TC, 512).transpose(1, 0, 2)).astype(bf16),
        "sind": np.ascontiguousarray(
            sin_np.reshape(128, TC, 512).transpose(1, 0, 2)).astype(bf16),
        "maskd": np.ascontiguousarray(mask_np).astype(bf16),
    }


def run(x, W_qkv, b_qkv, W_out, b_out, trace=False):
    global _compiled
    if _compiled is None:
        _compiled = _build()
    nc = _compiled

    shared = _shared_inputs()
    group_inp = [_prep_core_inputs(x, W_qkv, b_qkv, W_out, g) for g in range(2)]

    in_maps = []
    for core in range(N_CORES):
        b, g = core // 2, core % 2
        # [c4, p, ks, q]
        xT_np = (x[b].reshape(TC, 512, KSUB, 128).transpose(0, 3, 2, 1))
        m = {"xT": np.ascontiguousarray(xT_np).astype(bf16)}
        m.update(group_inp[g])
        m.update(shared)
        in_maps.append(m)

    res = run_bass_kernel_spmd(
        nc, in_maps, core_ids=list(range(N_CORES)), trace=trace,
        stitch_traces=trace,
    )
    outp = np.empty((B, T, C), dtype=np.float32)
    for b in range(B):
        outp[b] = (res.results[2 * b]["out"].astype(np.float32)
                   + res.results[2 * b + 1]["out"].astype(np.float32)
                   + b_out[None, :])
    return outp, res


def kernel(x, W_qkv, b_qkv, W_out, b_out):
    x = np.asarray(x, dtype=np.float32)
    W_qkv = np.asarray(W_qkv, dtype=np.float32)
    b_qkv = np.asarray(b_qkv, dtype=np.float32)
    W_out = np.asarray(W_out, dtype=np.float32)
    b_out = np.asarray(b_out, dtype=np.float32)
    outp, _ = run(x, W_qkv, b_qkv, W_out, b_out, trace=False)
    return outp



# revision 86
# speedup vs baseline: 1.0504x; 1.0504x over previous
"""Causal self-attention (B=4, T=2048, D=1024, H=16) on 8 TRN2 NeuronCores.

Sharding: core c handles batch b = c // 2 and head-group g = c % 2
(8 heads = 512 of the 1024 feature dims). Each core:
  1. QKV projection for its head-group's columns. q, k are produced
     TRANSPOSED ([feat, tok], feature dim on partitions) so they feed the
     attention matmuls directly; v is produced natural ([tok, feat]) so it
     is the PV stationary operand.
  2. RoPE via a PE rotation matmul (rotate_half as a constant 128x128
     block-diagonal permutation) + DVE combine with cos/sin.
  3. Causal attention with scores in [k, q] orientation: exp(score/8 - 2)
     without max-subtraction (shift-invariant), row-sum obtained free via a
     ones-column appended to v (PV matmul M=65: rows 0-63 = y, row 64 =
     softmax denominator).
  4. Late softmax normalization (reciprocal + gpsimd partition-broadcast),
     then the partial output projection with its 512 rows of W_out.
Host sums the two head-group partials per batch and adds b_out.

Schedule: token chunks (512 q each) outermost, head pairs inner; each
block's thin diagonal score tiles run first so their longer
scores->exp->PV chains overlap the dense tiles' PE work. Projection /
v / out-proj work items stream into the attention pipeline's PE gaps
under a per-chunk quota that saves the out-projections for the last
(exp-bound) chunk; the last chunk's out-proj splits its accumulation so
only the final head-pair matmuls wait on the last epilogue. DMA layouts
keep per-partition runs >= 512B (sub-512B runs pay 2x in the DMA
engines) and the startup loads alternate the two HWDGE queues in
consumption order.

All matmuls run in bf16 (fp32 matmul is 1/4 rate on the PE; fp8 PV
fails the accuracy gate); softmax statistics accumulate in fp32 PSUM.
"""

import numpy as np
import ml_dtypes

import concourse.tile as tile
from concourse import bacc, mybir
from concourse.bass_utils import run_bass_kernel_spmd

dt = mybir.dt
bf16 = ml_dtypes.bfloat16

B, T, C = 4, 2048, 1024
H, HD = 16, 64
N_CORES = 8
HPC = 8          # heads per core
FPC = H // 2 * HD // 8 * 8 // 2 * 2  # = 512 features per core (q, k, v each)
KSUB = C // 128  # 8 contraction subtiles
TT = T // 128    # 16 token tiles
TC = T // 512    # 4 token chunks

_compiled = None


def _build():
    nc = bacc.Bacc()
    dts = dt.bfloat16

    xT = nc.dram_tensor("xT", [TC, 128, KSUB, 512], dts, kind="ExternalInput")
    Wqk = nc.dram_tensor("Wqk", [8, 128, KSUB, 128], dts, kind="ExternalInput")
    Wv = nc.dram_tensor("Wv", [128, KSUB, 512], dts, kind="ExternalInput")
    Wo = nc.dram_tensor("Wo", [128, 4, 1024], dts, kind="ExternalInput")
    bqk = nc.dram_tensor("bqk", [128, 8], dt.float32, kind="ExternalInput")
    bv = nc.dram_tensor("bv", [128, 512], dt.float32, kind="ExternalInput")
    RT = nc.dram_tensor("RT", [128, 128], dts, kind="ExternalInput")
    cosd = nc.dram_tensor("cosd", [TC, 128, 512], dts, kind="ExternalInput")
    sind = nc.dram_tensor("sind", [TC, 128, 512], dts, kind="ExternalInput")
    maskd = nc.dram_tensor("maskd", [128, 128], dts, kind="ExternalInput")
    out = nc.dram_tensor("out", [T, C], dts, kind="ExternalOutput")

    with tile.TileContext(nc) as tc:
        with (
            tc.tile_pool(name="weights", bufs=1) as wp,
            tc.tile_pool(name="acts", bufs=1) as ap,
            tc.tile_pool(name="scratch", bufs=2) as sp,
            tc.tile_pool(name="exps", bufs=5) as ep,
            tc.tile_pool(name="norm", bufs=2) as np_,
            tc.tile_pool(name="outs", bufs=3) as op,
            tc.tile_pool(name="psum", bufs=2, space="PSUM") as pp,
            tc.tile_pool(name="psum_big", bufs=2, space="PSUM") as pb,
            tc.tile_pool(name="psum_pv", bufs=2, space="PSUM") as ppv,
        ):
            # chunk-major xT and fs-major Wqk: DMA destination runs are
            # 8KB/2KB contiguous per partition (sub-512B runs pay 2x in the
            # DMA engines)
            xT_sb = wp.tile([128, TC, KSUB, 512], dts)
            Wqk_sb = wp.tile([128, 8, KSUB, 128], dts)
            Wv_sb = wp.tile([128, KSUB, 512], dts)
            Wo_sb = wp.tile([128, 4, 1024], dts)
            bqk_sb = wp.tile([128, 8], dt.float32)
            bv_sb = wp.tile([128, 512], dt.float32)
            RT_sb = wp.tile([128, 128], dts)
            cos_sb = wp.tile([128, T], dts)
            sin_sb = wp.tile([128, T], dts)
            mask_sb = wp.tile([128, 128], dts)
            # exp bias constant (-2): keeps fp8e4m3 exp outputs under the
            # 448 max (softmax is shift-invariant, numerator and denominator
            # both scale by e^-2)
            nb2 = wp.tile([128, 1], dt.float32)
            nc.vector.memset(nb2[:], -2.0)

            def load_xT(c4):
                nc.sync.dma_start(xT_sb[:, c4], xT[c4])

            def load_wqk(fs):
                nc.sync.dma_start(Wqk_sb[:, fs], Wqk[fs])

            # first-needed data up front, in consumption order, alternating
            # the two HWDGE queues (SP + Activation) so descriptor issue
            # (fixed cost per DMA) pipelines with the transfers
            cosv = cos_sb.rearrange("p (c q) -> p c q", c=TC)
            sinv = sin_sb.rearrange("p (c q) -> p c q", c=TC)
            load_wqk(0)
            nc.scalar.dma_start(xT_sb[:, 0, 0:4], xT[0][:, 0:4, :])
            nc.scalar.dma_start(Wqk_sb[:, 4], Wqk[4])
            nc.sync.dma_start(xT_sb[:, 0, 4:8], xT[0][:, 4:8, :])
            nc.scalar.dma_start(bqk_sb[:], bqk[:])
            nc.sync.dma_start(cosv[:, 0], cosd[0])
            nc.scalar.dma_start(sinv[:, 0], sind[0])
            nc.sync.dma_start(RT_sb[:], RT[:])
            nc.sync.dma_start(mask_sb[:], maskd[:])
            load_wqk(1)
            nc.scalar.dma_start(Wqk_sb[:, 5], Wqk[5])
            load_wqk(2)
            nc.scalar.dma_start(Wqk_sb[:, 6], Wqk[6])
            load_wqk(3)
            nc.scalar.dma_start(Wqk_sb[:, 7], Wqk[7])
            nc.sync.dma_start(Wv_sb[:], Wv[:])
            nc.scalar.dma_start(bv_sb[:], bv[:])
            load_xT(1)
            nc.scalar.dma_start(cosv[:, 1], cosd[1])
            nc.scalar.dma_start(sinv[:, 1], sind[1])
            nc.sync.dma_start(Wo_sb[:], Wo[:])
            for c4 in range(2, TC):
                load_xT(c4)
                nc.sync.dma_start(cosv[:, c4], cosd[c4])
                nc.sync.dma_start(sinv[:, c4], sind[c4])

            qT_sb = ap.tile([128, 4, T], dts)   # rope'd q, [feat, tok]
            kT_sb = ap.tile([128, 4, T], dts)   # rope'd k, [feat, tok]
            # v natural + ones col per head (bf16 master copy for the
            # diagonal single-tile PVs)
            v_sb = ap.tile([128, TT, 8 * 65], dts)
            # error-compensated fp8 pair for the dense DoubleRow PVs:
            # v ~= v8h + v8e with both operands fp8e4m3 (DoubleRow needs
            # fp8 on both sides; the residual split reconstructs v to
            # ~0.1%). 68-wide head stride keeps the Ko step 16B-aligned.
            dt8 = dt.float8e4
            v8h = ap.tile([128, TT, 8 * 68], dt8)
            v8e = ap.tile([128, TT, 8 * 68], dt8)
            # normalized attention out (out-proj lhsT), one tile per token
            # chunk so out-proj of chunk c has no (tracker-level) dependency
            # on later chunks' y writes
            y_tiles = [ap.tile([128, 4, 512], dts, name=f"y{c}")
                       for c in range(TC)]

            # ones columns of v (col 64 of each head's block): 1.0 in the
            # master and the fp8-hi copy, 0.0 residual (1.0 is exact in fp8)
            v_heads = v_sb.rearrange("p t (h f) -> p t h f", h=8)
            v8h_heads = v8h.rearrange("p t (h f) -> p t h f", h=8)
            v8e_heads = v8e.rearrange("p t (h f) -> p t h f", h=8)
            nc.vector.memset(v_heads[:, :, :, 64], 1.0)
            nc.vector.memset(v8h_heads[:, :, :, 64], 1.0)
            nc.vector.memset(v8e_heads[:, :, :, 64], 0.0)

            # ---- fine-grained work emitters -----------------------------
            def v_tile(tt):
                psv = pp.tile([128, 512], dt.float32, tag="ps512")
                for ks in range(KSUB):
                    nc.tensor.matmul(
                        psv[:],
                        xT_sb[:, tt // 4, ks, (tt % 4) * 128:(tt % 4 + 1) * 128],
                        Wv_sb[:, ks, :],
                        start=(ks == 0), stop=(ks == KSUB - 1),
                    )
                nc.vector.tensor_add(
                    v_heads[:, tt, :, 0:64],
                    psv[:].rearrange("p (h f) -> p h f", h=8),
                    bv_sb[:].rearrange("p (h f) -> p h f", h=8),
                )
                # fp8 hi + residual split for the DoubleRow path
                nc.vector.tensor_copy(
                    v8h_heads[:, tt, :, 0:64], v_heads[:, tt, :, 0:64])
                nc.vector.tensor_sub(
                    v8e_heads[:, tt, :, 0:64],
                    v_heads[:, tt, :, 0:64], v8h_heads[:, tt, :, 0:64])

            def proj_rope(fs, c4, use_big=False):
                tsl = slice(c4 * 512, (c4 + 1) * 512)
                if use_big:
                    # bootstrap: attention pools are idle, borrow a big tile
                    bigt = pb.tile([128, 1024], dt.float32, tag="big")
                    ps, rps = bigt[:, 0:512], bigt[:, 512:1024]
                else:
                    # single tile: the rope matmul reuses ps once the bias
                    # extract has read it, so two projs pipeline through the
                    # two pp buffers instead of one
                    ps = pp.tile([128, 512], dt.float32, tag="ps512")
                    rps = ps
                for ks in range(KSUB):
                    nc.tensor.matmul(
                        ps[:],
                        Wqk_sb[:, fs, ks, :],
                        xT_sb[:, c4, ks, :],
                        start=(ks == 0), stop=(ks == KSUB - 1),
                    )
                qb = sp.tile([128, 512], dt.float32, tag="qb")
                nc.vector.tensor_scalar_add(qb[:], ps[:], bqk_sb[:, fs:fs + 1])
                u = sp.tile([128, 512], dts, tag="u")
                nc.vector.tensor_mul(u[:], qb[:], sin_sb[:, tsl])
                w = sp.tile([128, 512], dt.float32, tag="w")
                nc.vector.tensor_mul(w[:], qb[:], cos_sb[:, tsl])
                nc.tensor.matmul(rps[:], RT_sb[:], u[:], start=True, stop=True)
                dst = qT_sb if fs < 4 else kT_sb
                nc.vector.tensor_add(dst[:, fs % 4, tsl], w[:], rps[:])

            def out_proj(tt):
                yt = y_tiles[tt // 4]
                t0 = (tt % 4) * 128
                for n2 in range(2):
                    po = pp.tile([128, 512], dt.float32, tag="ps512")
                    for s in range(4):
                        nc.tensor.matmul(
                            po[:],
                            yt[:, s, t0:t0 + 128],
                            Wo_sb[:, s, n2 * 512:(n2 + 1) * 512],
                            start=(s == 0), stop=(s == 3),
                        )
                    ost = op.tile([128, 512], dts, tag="ost")
                    nc.vector.tensor_copy(ost[:], po[:])
                    nc.sync.dma_start(
                        out[tt * 128:(tt + 1) * 128, n2 * 512:(n2 + 1) * 512],
                        ost[:],
                    )

            from collections import deque

            # work items streamed into the attention pipeline's PE gaps.
            # Projections/v for chunk c must finish before chunk c's
            # attention; out-proj items are appended once a chunk's y is
            # final and are held back preferentially for the (long) last
            # chunk's j-loop, which otherwise runs dry and stalls on exp.
            fillers = deque()
            for c4 in range(1, TC):
                for fs in (0, 4, 1, 5, 2, 6, 3, 7):
                    fillers.append(("proj", fs, c4))
                for tt in range(4 * c4, 4 * c4 + 4):
                    fillers.append(("v", tt))

            def out_tail(tt):
                # last chunk's out-proj: accumulate the first three
                # head-pair contributions in a wide pb tile (hoistable ahead
                # of the final epilogue); only the s=3 matmuls and the copy
                # wait on the last y write
                yt = y_tiles[TC - 1]
                t0 = (tt % 4) * 128
                po2 = pb.tile([128, 1024], dt.float32, tag="big")
                for n2 in range(2):
                    for s in range(3):
                        nc.tensor.matmul(
                            po2[:, n2 * 512:(n2 + 1) * 512],
                            yt[:, s, t0:t0 + 128],
                            Wo_sb[:, s, n2 * 512:(n2 + 1) * 512],
                            start=(s == 0), stop=False,
                            skip_group_check=True,
                        )
                ost2 = op.tile([128, 1024], dts, tag="ost2")
                for n2 in range(2):
                    nc.tensor.matmul(
                        po2[:, n2 * 512:(n2 + 1) * 512],
                        yt[:, 3, t0:t0 + 128],
                        Wo_sb[:, 3, n2 * 512:(n2 + 1) * 512],
                        start=False, stop=True,
                        skip_group_check=True,
                    )
                    # copy each half right after its s=3 matmul so the copy
                    # train pipelines with the remaining tail matmuls
                    nc.vector.tensor_copy(
                        ost2[:, n2 * 512:(n2 + 1) * 512],
                        po2[:, n2 * 512:(n2 + 1) * 512])
                    deng = nc.scalar if (tt + n2) % 2 else nc.sync
                    deng.dma_start(
                        out[tt * 128:(tt + 1) * 128,
                            n2 * 512:(n2 + 1) * 512],
                        ost2[:, n2 * 512:(n2 + 1) * 512])

            def run_item(it):
                if it[0] == "v":
                    v_tile(it[1])
                elif it[0] == "proj":
                    proj_rope(it[1], it[2])
                elif it[0] == "outt":
                    out_tail(it[1])
                else:
                    out_proj(it[1])

            def drain_needed(qc):
                rest = deque()
                while fillers:
                    it = fillers.popleft()
                    if (it[0] == "proj" and it[2] <= qc) or \
                       (it[0] == "v" and it[1] < 4 * qc + 4):
                        run_item(it)
                    else:
                        rest.append(it)
                fillers.extend(rest)

            # bootstrap: all head pairs' q/k for chunk 0 + v for chunk 0
            for fs in (0, 4, 1, 5, 2, 6, 3, 7):
                proj_rope(fs, 0, use_big=True)
            for tt in range(4):
                v_tile(tt)

            for qc in range(TC):
                if qc:
                    drain_needed(qc)
                iters = 4 * (4 * qc + 4)
                # quota: earlier chunks emit only the next chunk's deps
                # (their attention windows are PE-bound); all out-proj work
                # is held for the last chunk's ACT-bound window, minus two
                # items kept past the loop to cover the final epilogue
                if qc == TC - 1:
                    quota = max(0, len(fillers) - 2)
                else:
                    quota = min(len(fillers), 12)
                pops = it_count = 0
                for hp in range(4):
                    qsl = slice(qc * 512, (qc + 1) * 512)
                    jmax = 4 * qc + 3
                    pv0 = ppv.tile([65, 512], dt.float32, tag="pv")
                    pv1 = ppv.tile([65, 512], dt.float32, tag="pv")
                    # diagonal (thin) tiles first as bf16 singles (their
                    # longer chains overlap the dense work); dense tiles as
                    # error-compensated fp8 DoubleRow pairs — two k-blocks
                    # per matmul at half cycles-per-row, two matmuls (hi +
                    # residual) reconstructing v to ~0.1%
                    def scores_j(j, qs):
                        big = pb.tile([128, 1024], dt.float32, tag="big")
                        for par in range(2):
                            kb = par * 64
                            o = par * 512
                            nc.tensor.matmul(
                                big[:, o + qs:o + 512],
                                kT_sb[kb:kb + 64, hp, j * 128:(j + 1) * 128],
                                qT_sb[kb:kb + 64, hp,
                                      qc * 512 + qs:(qc + 1) * 512],
                                start=True, stop=True,
                            )
                        return big.rearrange("p (two q) -> p two q", two=2)

                    parts = [("s", 4 * qc + d) for d in range(4)]
                    parts += [("d", 2 * m) for m in range(2 * qc)]
                    for pi, (kind, j0) in enumerate(parts):
                        last_part = pi == len(parts) - 1
                        if kind == "s":
                            qs = j0 * 128 - qc * 512
                            big_v = scores_j(j0, qs)
                            ex = ep.tile([128, 1024], dts, tag="ex")
                            ex_v = ex.rearrange("p (two q) -> p two q", two=2)
                            nc.scalar.activation(
                                ex_v[:, :, qs:512], big_v[:, :, qs:512],
                                mybir.ActivationFunctionType.Exp,
                                bias=nb2[:], scale=0.125,
                            )
                            # diagonal tile: zero the strictly-upper part
                            nc.vector.tensor_tensor(
                                ex_v[:, :, qs:qs + 128],
                                ex_v[:, :, qs:qs + 128],
                                mask_sb[:, None, :].to_broadcast((128, 2, 128)),
                                mybir.AluOpType.mult,
                            )
                            for par in range(2):
                                h = 2 * hp + par
                                pv = pv0 if par == 0 else pv1
                                nc.tensor.matmul(
                                    pv[:, qs:512],
                                    v_sb[:, j0, h * 65:(h + 1) * 65],
                                    ex[:, par * 512 + qs:par * 512 + 512],
                                    start=(pi == 0), stop=last_part,
                                    skip_group_check=True,
                                )
                            it_count += 1
                        else:
                            ex2 = ep.tile([128, 2, 1024], dt8, tag="ex2")
                            for i in range(2):
                                big_v = scores_j(j0 + i, 0)
                                nc.scalar.activation(
                                    ex2[:, i].rearrange(
                                        "p (two q) -> p two q", two=2)[:, :, :],
                                    big_v[:, :, :],
                                    mybir.ActivationFunctionType.Exp,
                                    bias=nb2[:], scale=0.125,
                                )
                            for par in range(2):
                                h = 2 * hp + par
                                pv = pv0 if par == 0 else pv1
                                for v8 in (v8h, v8e):
                                    nc.tensor.matmul(
                                        pv[:, 0:512],
                                        v8[:, j0:j0 + 2, h * 68:h * 68 + 65],
                                        ex2[:, :, par * 512:par * 512 + 512],
                                        start=False,
                                        stop=(last_part and v8 is v8e),
                                        skip_group_check=True,
                                        perf_mode=mybir.MatmulPerfMode.DoubleRow,
                                    )
                            it_count += 2
                        while (not last_part and fillers and pops < quota
                               and pops * iters < quota * it_count):
                            run_item(fillers.popleft())
                            pops += 1
                    last = qc == TC - 1 and hp == 3
                    for par in range(2):
                        h = 2 * hp + par
                        kb = par * 64
                        pv = pv0 if par == 0 else pv1
                        if not last and qc >= 2:
                            # free the pv PSUM bank fast (the next block's
                            # first PV reuses it): evacuate to SBUF, then
                            # normalize from the copy
                            pvc = np_.tile([65, 512], dt.float32, tag="pvc")
                            nc.vector.tensor_copy(pvc[:], pv[:])
                            pv = pvc
                        rinv = np_.tile([1, 512], dt.float32, tag="rinv")
                        nc.vector.reciprocal(rinv[0:1, :], pv[64:65, :])
                        rb = np_.tile([64, 512], dt.float32, tag="rb")
                        nc.gpsimd.partition_broadcast(rb[:], rinv[0:1, :])
                        nc.vector.tensor_mul(
                            y_tiles[qc][kb:kb + 64, hp, :], pv[0:64, :], rb[:],
                        )
                    while (fillers and pops < quota
                           and pops * iters < quota * it_count):
                        run_item(fillers.popleft())
                        pops += 1
                    if hp == 3:
                        # this chunk's y is final for all heads: stream out-proj
                        kind = "outt" if qc == TC - 1 else "out"
                        for tt in range(4 * qc, 4 * qc + 4):
                            fillers.append((kind, tt))
            while fillers:
                run_item(fillers.popleft())

    nc.compile()
    return nc


def _prep_core_inputs(x, W_qkv, b_qkv, W_out, g):
    """Host-side shard prep for head-group g (features g*512:(g+1)*512)."""
    fs = slice(g * 512, (g + 1) * 512)
    Wq = W_qkv[:, 0:1024][:, fs]          # [1024, 512]
    Wk = W_qkv[:, 1024:2048][:, fs]
    Wv_ = W_qkv[:, 2048:3072][:, fs]
    bq = b_qkv[0:1024][fs]
    bk = b_qkv[1024:2048][fs]
    bv_ = b_qkv[2048:3072][fs]

    Wqk_np = np.concatenate([Wq, Wk], axis=1)        # [1024, 1024]
    # [fs, p, ks, col]
    Wqk_np = Wqk_np.reshape(KSUB, 128, 8, 128).transpose(2, 1, 0, 3)
    Wv_np = Wv_.reshape(KSUB, 128, 512).transpose(1, 0, 2)
    Wo_np = W_out[fs, :].reshape(4, 128, 1024).transpose(1, 0, 2)
    bqk_np = np.concatenate([bq, bk]).reshape(8, 128).T.copy()   # [128, 8]
    bv_np = np.broadcast_to(bv_[None, :], (128, 512)).copy()

    return {
        "Wqk": np.ascontiguousarray(Wqk_np).astype(bf16),
        "Wv": np.ascontiguousarray(Wv_np).astype(bf16),
        "Wo": np.ascontiguousarray(Wo_np).astype(bf16),
        "bqk": np.ascontiguousarray(bqk_np).astype(np.float32),
        "bv": bv_np.astype(np.float32),
    }


def _shared_inputs():
    # rotation matrix: (R q)[d] = -q[d+32] for d<32, q[d-32] for 32<=d<64
    R64 = np.zeros((64, 64), dtype=np.float32)
    for d in range(32):
        R64[d, d + 32] = -1.0
        R64[d + 32, d] = 1.0
    R128 = np.zeros((128, 128), dtype=np.float32)
    R128[0:64, 0:64] = R64
    R128[64:128, 64:128] = R64
    RT_np = R128.T.copy()

    inv_freq = 1.0 / (10000.0 ** (np.arange(0, HD, 2, dtype=np.float32) / HD))
    t = np.arange(T, dtype=np.float32)
    freqs = np.outer(t, inv_freq)                     # [T, 32]
    p = np.arange(128)
    cos_np = np.cos(freqs[:, p % 32]).T.copy()        # [128, T]
    sin_np = np.sin(freqs[:, p % 32]).T.copy()

    # causal mask for the diagonal 128-block: mask[k, q] = 1 iff k <= q
    mask_np = np.tril(np.ones((128, 128), dtype=np.float32)).T.copy()

    return {
        "RT": RT_np.astype(bf16),
        "cosd": np.ascontiguousarray(
            cos_np.reshape(128, TC, 512).transpose(1, 0, 2)).astype(bf16),
        "sind": np.ascontiguousarray(
            sin_np.reshape(128, TC, 512).transpose(1, 0, 2)).astype(bf16),
        "maskd": np.ascontiguousarray(mask_np).astype(bf16),
    }


def run(x, W_qkv, b_qkv, W_out, b_out, trace=False):
    global _compiled
    if _compiled is None:
        _compiled = _build()
    nc = _compiled

    shared = _shared_inputs()
    group_inp = [_prep_core_inputs(x, W_qkv, b_qkv, W_out, g) for g in range(2)]

    in_maps = []
    for core in range(N_CORES):
        b, g = core // 2, core % 2
        # [c4, p, ks, q]
        xT_np = (x[b].reshape(TC, 512, KSUB, 128).transpose(0, 3, 2, 1))
        m = {"xT": np.ascontiguousarray(xT_np).astype(bf16)}
        m.update(group_inp[g])
        m.update(shared)
        in_maps.append(m)

    res = run_bass_kernel_spmd(
        nc, in_maps, core_ids=list(range(N_CORES)), trace=trace,
        stitch_traces=trace,
    )
    outp = np.empty((B, T, C), dtype=np.float32)
    for b in range(B):
        outp[b] = (res.results[2 * b]["out"].astype(np.float32)
                   + res.results[2 * b + 1]["out"].astype(np.float32)
                   + b_out[None, :])
    return outp, res


def kernel(x, W_qkv, b_qkv, W_out, b_out):
    x = np.asarray(x, dtype=np.float32)
    W_qkv = np.asarray(W_qkv, dtype=np.float32)
    b_qkv = np.asarray(b_qkv, dtype=np.float32)
    W_out = np.asarray(W_out, dtype=np.float32)
    b_out = np.asarray(b_out, dtype=np.float32)
    outp, _ = run(x, W_qkv, b_qkv, W_out, b_out, trace=False)
    return outp



# revision 90
# speedup vs baseline: 1.0520x; 1.0015x over previous
"""Causal self-attention (B=4, T=2048, D=1024, H=16) on 8 TRN2 NeuronCores.

Sharding: core c handles batch b = c // 2 and head-group g = c % 2
(8 heads = 512 of the 1024 feature dims). Each core:
  1. QKV projection for its head-group's columns. q, k are produced
     TRANSPOSED ([feat, tok], feature dim on partitions) so they feed the
     attention matmuls directly; v is produced natural ([tok, feat]) so it
     is the PV stationary operand.
  2. RoPE via a PE rotation matmul (rotate_half as a constant 128x128
     block-diagonal permutation) + DVE combine with cos/sin.
  3. Causal attention with scores in [k, q] orientation: exp(score/8 - 2)
     without max-subtraction (shift-invariant), row-sum obtained free via a
     ones-column appended to v (PV matmul M=65: rows 0-63 = y, row 64 =
     softmax denominator).
  4. Late softmax normalization (reciprocal + gpsimd partition-broadcast),
     then the partial output projection with its 512 rows of W_out.
Host sums the two head-group partials per batch and adds b_out.

Schedule: token chunks (512 q each) outermost, head pairs inner; each
block's thin diagonal score tiles run first so their longer
scores->exp->PV chains overlap the dense tiles' PE work. Projection /
v / out-proj work items stream into the attention pipeline's PE gaps
under a per-chunk quota that saves the out-projections for the last
(exp-bound) chunk; the last chunk's out-proj splits its accumulation so
only the final head-pair matmuls wait on the last epilogue. DMA layouts
keep per-partition runs >= 512B (sub-512B runs pay 2x in the DMA
engines) and the startup loads alternate the two HWDGE queues in
consumption order.

Projections and scores run in bf16 (fp32 matmul is 1/4 rate on the
PE). The dense PV tiles run as error-compensated fp8e4m3 DoubleRow
pairs: two k-blocks contract per matmul at half cycles-per-row, and
v is split hi + residual (two DoubleRow matmuls — still half the bf16
cost) so v reconstructs to ~0.1%; plain fp8 v fails the accuracy gate
because concentrated attention passes its quantization error straight
through. Softmax statistics accumulate in fp32 PSUM.
"""

import numpy as np
import ml_dtypes

import concourse.tile as tile
from concourse import bacc, mybir
from concourse.bass_utils import run_bass_kernel_spmd

dt = mybir.dt
bf16 = ml_dtypes.bfloat16

B, T, C = 4, 2048, 1024
H, HD = 16, 64
N_CORES = 8
HPC = 8          # heads per core
FPC = H // 2 * HD // 8 * 8 // 2 * 2  # = 512 features per core (q, k, v each)
KSUB = C // 128  # 8 contraction subtiles
TT = T // 128    # 16 token tiles
TC = T // 512    # 4 token chunks

_compiled = None


def _build():
    nc = bacc.Bacc()
    dts = dt.bfloat16

    xT = nc.dram_tensor("xT", [TC, 128, KSUB, 512], dts, kind="ExternalInput")
    Wqk = nc.dram_tensor("Wqk", [8, 128, KSUB, 128], dts, kind="ExternalInput")
    Wv = nc.dram_tensor("Wv", [128, KSUB, 512], dts, kind="ExternalInput")
    Wo = nc.dram_tensor("Wo", [128, 4, 1024], dts, kind="ExternalInput")
    bqk = nc.dram_tensor("bqk", [128, 8], dt.float32, kind="ExternalInput")
    bv = nc.dram_tensor("bv", [128, 512], dt.float32, kind="ExternalInput")
    RT = nc.dram_tensor("RT", [128, 128], dts, kind="ExternalInput")
    cosd = nc.dram_tensor("cosd", [TC, 128, 512], dts, kind="ExternalInput")
    sind = nc.dram_tensor("sind", [TC, 128, 512], dts, kind="ExternalInput")
    maskd = nc.dram_tensor("maskd", [128, 128], dts, kind="ExternalInput")
    out = nc.dram_tensor("out", [T, C], dts, kind="ExternalOutput")

    with tile.TileContext(nc) as tc:
        with (
            tc.tile_pool(name="weights", bufs=1) as wp,
            tc.tile_pool(name="acts", bufs=1) as ap,
            tc.tile_pool(name="scratch", bufs=2) as sp,
            tc.tile_pool(name="exps", bufs=5) as ep,
            tc.tile_pool(name="norm", bufs=2) as np_,
            tc.tile_pool(name="outs", bufs=3) as op,
            tc.tile_pool(name="psum", bufs=2, space="PSUM") as pp,
            tc.tile_pool(name="psum_big", bufs=2, space="PSUM") as pb,
            tc.tile_pool(name="psum_pv", bufs=2, space="PSUM") as ppv,
        ):
            # chunk-major xT and fs-major Wqk: DMA destination runs are
            # 8KB/2KB contiguous per partition (sub-512B runs pay 2x in the
            # DMA engines)
            xT_sb = wp.tile([128, TC, KSUB, 512], dts)
            Wqk_sb = wp.tile([128, 8, KSUB, 128], dts)
            Wv_sb = wp.tile([128, KSUB, 512], dts)
            Wo_sb = wp.tile([128, 4, 1024], dts)
            bqk_sb = wp.tile([128, 8], dt.float32)
            bv_sb = wp.tile([128, 512], dt.float32)
            RT_sb = wp.tile([128, 128], dts)
            cos_sb = wp.tile([128, T], dts)
            sin_sb = wp.tile([128, T], dts)
            mask_sb = wp.tile([128, 128], dts)
            # exp bias constant (-2): keeps fp8e4m3 exp outputs under the
            # 448 max (softmax is shift-invariant, numerator and denominator
            # both scale by e^-2)
            nb2 = wp.tile([128, 1], dt.float32)
            nc.vector.memset(nb2[:], -2.0)

            def load_xT(c4):
                nc.sync.dma_start(xT_sb[:, c4], xT[c4])

            def load_wqk(fs):
                nc.sync.dma_start(Wqk_sb[:, fs], Wqk[fs])

            # first-needed data up front, in consumption order, alternating
            # the two HWDGE queues (SP + Activation) so descriptor issue
            # (fixed cost per DMA) pipelines with the transfers
            cosv = cos_sb.rearrange("p (c q) -> p c q", c=TC)
            sinv = sin_sb.rearrange("p (c q) -> p c q", c=TC)
            load_wqk(0)
            nc.scalar.dma_start(xT_sb[:, 0, 0:4], xT[0][:, 0:4, :])
            nc.scalar.dma_start(Wqk_sb[:, 4], Wqk[4])
            nc.sync.dma_start(xT_sb[:, 0, 4:8], xT[0][:, 4:8, :])
            nc.scalar.dma_start(bqk_sb[:], bqk[:])
            nc.sync.dma_start(cosv[:, 0], cosd[0])
            nc.scalar.dma_start(sinv[:, 0], sind[0])
            nc.sync.dma_start(RT_sb[:], RT[:])
            nc.sync.dma_start(mask_sb[:], maskd[:])
            load_wqk(1)
            nc.scalar.dma_start(Wqk_sb[:, 5], Wqk[5])
            load_wqk(2)
            nc.scalar.dma_start(Wqk_sb[:, 6], Wqk[6])
            load_wqk(3)
            nc.scalar.dma_start(Wqk_sb[:, 7], Wqk[7])
            nc.sync.dma_start(Wv_sb[:], Wv[:])
            nc.scalar.dma_start(bv_sb[:], bv[:])
            load_xT(1)
            nc.scalar.dma_start(cosv[:, 1], cosd[1])
            nc.scalar.dma_start(sinv[:, 1], sind[1])
            nc.sync.dma_start(Wo_sb[:], Wo[:])
            for c4 in range(2, TC):
                load_xT(c4)
                nc.sync.dma_start(cosv[:, c4], cosd[c4])
                nc.sync.dma_start(sinv[:, c4], sind[c4])

            qT_sb = ap.tile([128, 4, T], dts)   # rope'd q, [feat, tok]
            kT_sb = ap.tile([128, 4, T], dts)   # rope'd k, [feat, tok]
            # v natural + ones col per head (bf16 master copy for the
            # diagonal single-tile PVs)
            v_sb = ap.tile([128, TT, 8 * 65], dts)
            # error-compensated fp8 pair for the dense DoubleRow PVs:
            # v ~= v8h + v8e with both operands fp8e4m3 (DoubleRow needs
            # fp8 on both sides; the residual split reconstructs v to
            # ~0.1%). 68-wide head stride keeps the Ko step 16B-aligned.
            dt8 = dt.float8e4
            v8h = ap.tile([128, TT, 8 * 68], dt8)
            v8e = ap.tile([128, TT, 8 * 68], dt8)
            # normalized attention out (out-proj lhsT), one tile per token
            # chunk so out-proj of chunk c has no (tracker-level) dependency
            # on later chunks' y writes
            y_tiles = [ap.tile([128, 4, 512], dts, name=f"y{c}")
                       for c in range(TC)]

            # ones columns of v (col 64 of each head's block): 1.0 in the
            # master and the fp8-hi copy, 0.0 residual (1.0 is exact in fp8)
            v_heads = v_sb.rearrange("p t (h f) -> p t h f", h=8)
            v8h_heads = v8h.rearrange("p t (h f) -> p t h f", h=8)
            v8e_heads = v8e.rearrange("p t (h f) -> p t h f", h=8)
            nc.vector.memset(v_heads[:, :, :, 64], 1.0)
            nc.vector.memset(v8h_heads[:, :, :, 64], 1.0)
            nc.vector.memset(v8e_heads[:, :, :, 64], 0.0)

            # ---- fine-grained work emitters -----------------------------
            def v_tile(tt):
                psv = pp.tile([128, 512], dt.float32, tag="ps512")
                for ks in range(KSUB):
                    nc.tensor.matmul(
                        psv[:],
                        xT_sb[:, tt // 4, ks, (tt % 4) * 128:(tt % 4 + 1) * 128],
                        Wv_sb[:, ks, :],
                        start=(ks == 0), stop=(ks == KSUB - 1),
                    )
                nc.vector.tensor_add(
                    v_heads[:, tt, :, 0:64],
                    psv[:].rearrange("p (h f) -> p h f", h=8),
                    bv_sb[:].rearrange("p (h f) -> p h f", h=8),
                )
                # fp8 hi + residual split for the DoubleRow path
                nc.vector.tensor_copy(
                    v8h_heads[:, tt, :, 0:64], v_heads[:, tt, :, 0:64])
                nc.vector.tensor_sub(
                    v8e_heads[:, tt, :, 0:64],
                    v_heads[:, tt, :, 0:64], v8h_heads[:, tt, :, 0:64])

            def proj_rope(fs, c4, use_big=False):
                tsl = slice(c4 * 512, (c4 + 1) * 512)
                if use_big:
                    # bootstrap: attention pools are idle, borrow a big tile
                    bigt = pb.tile([128, 1024], dt.float32, tag="big")
                    ps, rps = bigt[:, 0:512], bigt[:, 512:1024]
                else:
                    # single tile: the rope matmul reuses ps once the bias
                    # extract has read it, so two projs pipeline through the
                    # two pp buffers instead of one
                    ps = pp.tile([128, 512], dt.float32, tag="ps512")
                    rps = ps
                for ks in range(KSUB):
                    nc.tensor.matmul(
                        ps[:],
                        Wqk_sb[:, fs, ks, :],
                        xT_sb[:, c4, ks, :],
                        start=(ks == 0), stop=(ks == KSUB - 1),
                    )
                qb = sp.tile([128, 512], dt.float32, tag="qb")
                nc.vector.tensor_scalar_add(qb[:], ps[:], bqk_sb[:, fs:fs + 1])
                u = sp.tile([128, 512], dts, tag="u")
                nc.vector.tensor_mul(u[:], qb[:], sin_sb[:, tsl])
                w = sp.tile([128, 512], dt.float32, tag="w")
                nc.vector.tensor_mul(w[:], qb[:], cos_sb[:, tsl])
                nc.tensor.matmul(rps[:], RT_sb[:], u[:], start=True, stop=True)
                dst = qT_sb if fs < 4 else kT_sb
                nc.vector.tensor_add(dst[:, fs % 4, tsl], w[:], rps[:])

            def out_proj(tt):
                yt = y_tiles[tt // 4]
                t0 = (tt % 4) * 128
                for n2 in range(2):
                    po = pp.tile([128, 512], dt.float32, tag="ps512")
                    for s in range(4):
                        nc.tensor.matmul(
                            po[:],
                            yt[:, s, t0:t0 + 128],
                            Wo_sb[:, s, n2 * 512:(n2 + 1) * 512],
                            start=(s == 0), stop=(s == 3),
                        )
                    ost = op.tile([128, 512], dts, tag="ost")
                    nc.vector.tensor_copy(ost[:], po[:])
                    nc.sync.dma_start(
                        out[tt * 128:(tt + 1) * 128, n2 * 512:(n2 + 1) * 512],
                        ost[:],
                    )

            from collections import deque

            # work items streamed into the attention pipeline's PE gaps.
            # Projections/v for chunk c must finish before chunk c's
            # attention; out-proj items are appended once a chunk's y is
            # final and are held back preferentially for the (long) last
            # chunk's j-loop, which otherwise runs dry and stalls on exp.
            fillers = deque()
            for c4 in range(1, TC):
                for fs in (0, 4, 1, 5, 2, 6, 3, 7):
                    fillers.append(("proj", fs, c4))
                for tt in range(4 * c4, 4 * c4 + 4):
                    fillers.append(("v", tt))

            def out_tail(tt):
                # last chunk's out-proj: accumulate the first three
                # head-pair contributions in a wide pb tile (hoistable ahead
                # of the final epilogue); only the s=3 matmuls and the copy
                # wait on the last y write
                yt = y_tiles[TC - 1]
                t0 = (tt % 4) * 128
                po2 = pb.tile([128, 1024], dt.float32, tag="big")
                for n2 in range(2):
                    for s in range(3):
                        nc.tensor.matmul(
                            po2[:, n2 * 512:(n2 + 1) * 512],
                            yt[:, s, t0:t0 + 128],
                            Wo_sb[:, s, n2 * 512:(n2 + 1) * 512],
                            start=(s == 0), stop=False,
                            skip_group_check=True,
                        )
                ost2 = op.tile([128, 1024], dts, tag="ost2")
                for n2 in range(2):
                    nc.tensor.matmul(
                        po2[:, n2 * 512:(n2 + 1) * 512],
                        yt[:, 3, t0:t0 + 128],
                        Wo_sb[:, 3, n2 * 512:(n2 + 1) * 512],
                        start=False, stop=True,
                        skip_group_check=True,
                    )
                    # copy each half right after its s=3 matmul so the copy
                    # train pipelines with the remaining tail matmuls
                    nc.vector.tensor_copy(
                        ost2[:, n2 * 512:(n2 + 1) * 512],
                        po2[:, n2 * 512:(n2 + 1) * 512])
                    deng = nc.scalar if (tt + n2) % 2 else nc.sync
                    deng.dma_start(
                        out[tt * 128:(tt + 1) * 128,
                            n2 * 512:(n2 + 1) * 512],
                        ost2[:, n2 * 512:(n2 + 1) * 512])

            def run_item(it):
                if it[0] == "v":
                    v_tile(it[1])
                elif it[0] == "proj":
                    proj_rope(it[1], it[2])
                elif it[0] == "outt":
                    out_tail(it[1])
                else:
                    out_proj(it[1])

            def drain_needed(qc):
                rest = deque()
                while fillers:
                    it = fillers.popleft()
                    if (it[0] == "proj" and it[2] <= qc) or \
                       (it[0] == "v" and it[1] < 4 * qc + 4):
                        run_item(it)
                    else:
                        rest.append(it)
                fillers.extend(rest)

            # bootstrap: all head pairs' q/k for chunk 0 + v for chunk 0
            for fs in (0, 4, 1, 5, 2, 6, 3, 7):
                proj_rope(fs, 0, use_big=True)
            for tt in range(4):
                v_tile(tt)

            for qc in range(TC):
                if qc:
                    drain_needed(qc)
                iters = 4 * (4 * qc + 4)
                # quota: earlier chunks emit only the next chunk's deps
                # (their attention windows are PE-bound); all out-proj work
                # is held for the last chunk's ACT-bound window, minus two
                # items kept past the loop to cover the final epilogue
                if qc == TC - 1:
                    quota = max(0, len(fillers) - 2)
                else:
                    quota = min(len(fillers), 10)
                pops = it_count = 0
                for hp in range(4):
                    qsl = slice(qc * 512, (qc + 1) * 512)
                    jmax = 4 * qc + 3
                    pv0 = ppv.tile([65, 512], dt.float32, tag="pv")
                    pv1 = ppv.tile([65, 512], dt.float32, tag="pv")
                    # diagonal (thin) tiles first as bf16 singles (their
                    # longer chains overlap the dense work); dense tiles as
                    # error-compensated fp8 DoubleRow pairs — two k-blocks
                    # per matmul at half cycles-per-row, two matmuls (hi +
                    # residual) reconstructing v to ~0.1%
                    def scores_j(j, qs):
                        big = pb.tile([128, 1024], dt.float32, tag="big")
                        for par in range(2):
                            kb = par * 64
                            o = par * 512
                            nc.tensor.matmul(
                                big[:, o + qs:o + 512],
                                kT_sb[kb:kb + 64, hp, j * 128:(j + 1) * 128],
                                qT_sb[kb:kb + 64, hp,
                                      qc * 512 + qs:(qc + 1) * 512],
                                start=True, stop=True,
                            )
                        return big.rearrange("p (two q) -> p two q", two=2)

                    parts = [("s", 4 * qc + d) for d in range(4)]
                    parts += [("d", 2 * m) for m in range(2 * qc)]
                    for pi, (kind, j0) in enumerate(parts):
                        last_part = pi == len(parts) - 1
                        if kind == "s":
                            qs = j0 * 128 - qc * 512
                            big_v = scores_j(j0, qs)
                            ex = ep.tile([128, 1024], dts, tag="ex")
                            ex_v = ex.rearrange("p (two q) -> p two q", two=2)
                            nc.scalar.activation(
                                ex_v[:, :, qs:512], big_v[:, :, qs:512],
                                mybir.ActivationFunctionType.Exp,
                                bias=nb2[:], scale=0.125,
                            )
                            # diagonal tile: zero the strictly-upper part
                            nc.vector.tensor_tensor(
                                ex_v[:, :, qs:qs + 128],
                                ex_v[:, :, qs:qs + 128],
                                mask_sb[:, None, :].to_broadcast((128, 2, 128)),
                                mybir.AluOpType.mult,
                            )
                            for par in range(2):
                                h = 2 * hp + par
                                pv = pv0 if par == 0 else pv1
                                nc.tensor.matmul(
                                    pv[:, qs:512],
                                    v_sb[:, j0, h * 65:(h + 1) * 65],
                                    ex[:, par * 512 + qs:par * 512 + 512],
                                    start=(pi == 0), stop=last_part,
                                    skip_group_check=True,
                                )
                            it_count += 1
                        else:
                            ex2 = ep.tile([128, 2, 1024], dt8, tag="ex2")
                            for i in range(2):
                                big_v = scores_j(j0 + i, 0)
                                nc.scalar.activation(
                                    ex2[:, i].rearrange(
                                        "p (two q) -> p two q", two=2)[:, :, :],
                                    big_v[:, :, :],
                                    mybir.ActivationFunctionType.Exp,
                                    bias=nb2[:], scale=0.125,
                                )
                            for par in range(2):
                                h = 2 * hp + par
                                pv = pv0 if par == 0 else pv1
                                for v8 in (v8h, v8e):
                                    nc.tensor.matmul(
                                        pv[:, 0:512],
                                        v8[:, j0:j0 + 2, h * 68:h * 68 + 65],
                                        ex2[:, :, par * 512:par * 512 + 512],
                                        start=False,
                                        stop=(last_part and v8 is v8e),
                                        skip_group_check=True,
                                        perf_mode=mybir.MatmulPerfMode.DoubleRow,
                                    )
                            it_count += 2
                        while (not last_part and fillers and pops < quota
                               and pops * iters < quota * it_count):
                            run_item(fillers.popleft())
                            pops += 1
                    last = qc == TC - 1 and hp == 3
                    for par in range(2):
                        h = 2 * hp + par
                        kb = par * 64
                        pv = pv0 if par == 0 else pv1
                        if not last and qc >= 2:
                            # free the pv PSUM bank fast (the next block's
                            # first PV reuses it): evacuate to SBUF, then
                            # normalize from the copy
                            pvc = np_.tile([65, 512], dt.float32, tag="pvc")
                            nc.vector.tensor_copy(pvc[:], pv[:])
                            pv = pvc
                        rinv = np_.tile([1, 512], dt.float32, tag="rinv")
                        nc.vector.reciprocal(rinv[0:1, :], pv[64:65, :])
                        rb = np_.tile([64, 512], dt.float32, tag="rb")
                        nc.gpsimd.partition_broadcast(rb[:], rinv[0:1, :])
                        nc.vector.tensor_mul(
                            y_tiles[qc][kb:kb + 64, hp, :], pv[0:64, :], rb[:],
                        )
                    while (fillers and pops < quota
                           and pops * iters < quota * it_count):
                        run_item(fillers.popleft())
                        pops += 1
                    if hp == 3:
                        # this chunk's y is final for all heads: stream out-proj
                        kind = "outt" if qc == TC - 1 else "out"
                        for tt in range(4 * qc, 4 * qc + 4):
                            fillers.append((kind, tt))
            while fillers:
                run_item(fillers.popleft())

    nc.compile()
    return nc


def _prep_core_inputs(x, W_qkv, b_qkv, W_out, g):
    """Host-side shard prep for head-group g (features g*512:(g+1)*512)."""
    fs = slice(g * 512, (g + 1) * 512)
    Wq = W_qkv[:, 0:1024][:, fs]          # [1024, 512]
    Wk = W_qkv[:, 1024:2048][:, fs]
    Wv_ = W_qkv[:, 2048:3072][:, fs]
    bq = b_qkv[0:1024][fs]
    bk = b_qkv[1024:2048][fs]
    bv_ = b_qkv[2048:3072][fs]

    Wqk_np = np.concatenate([Wq, Wk], axis=1)        # [1024, 1024]
    # [fs, p, ks, col]
    Wqk_np = Wqk_np.reshape(KSUB, 128, 8, 128).transpose(2, 1, 0, 3)
    Wv_np = Wv_.reshape(KSUB, 128, 512).transpose(1, 0, 2)
    Wo_np = W_out[fs, :].reshape(4, 128, 1024).transpose(1, 0, 2)
    bqk_np = np.concatenate([bq, bk]).reshape(8, 128).T.copy()   # [128, 8]
    bv_np = np.broadcast_to(bv_[None, :], (128, 512)).copy()

    return {
        "Wqk": np.ascontiguousarray(Wqk_np).astype(bf16),
        "Wv": np.ascontiguousarray(Wv_np).astype(bf16),
        "Wo": np.ascontiguousarray(Wo_np).astype(bf16),
        "bqk": np.ascontiguousarray(bqk_np).astype(np.float32),
        "bv": bv_np.astype(np.float32),
    }


def _shared_inputs():
    # rotation matrix: (R q)[d] = -q[d+32] for d<32, q[d-32] for 32<=d<64
    R64 = np.zeros((64, 64), dtype=np.float32)
    for d in range(32):
        R64[d, d + 32] = -1.0
        R64[d + 32, d] = 1.0
    R128 = np.zeros((128, 128), dtype=np.float32)
    R128[0:64, 0:64] = R64
    R128[64:128, 64:128] = R64
    RT_np = R128.T.copy()

    inv_freq = 1.0 / (10000.0 ** (np.arange(0, HD, 2, dtype=np.float32) / HD))
    t = np.arange(T, dtype=np.float32)
    freqs = np.outer(t, inv_freq)                     # [T, 32]
    p = np.arange(128)
    cos_np = np.cos(freqs[:, p % 32]).T.copy()        # [128, T]
    sin_np = np.sin(freqs[:, p % 32]).T.copy()

    # causal mask for the diagonal 128-block: mask[k, q] = 1 iff k <= q
    mask_np = np.tril(np.ones((128, 128), dtype=np.float32)).T.copy()

    return {
        "RT": RT_np.astype(bf16),
        "cosd": np.ascontiguousarray(
            cos_np.reshape(128, TC, 512).transpose(1, 0, 2)).astype(bf16),
        "sind": np.ascontiguousarray(
            sin_np.reshape(128, TC, 512).transpose(1, 0, 2)).astype(bf16),
        "maskd": np.ascontiguousarray(mask_np).astype(bf16),
    }


def run(x, W_qkv, b_qkv, W_out, b_out, trace=False):
    global _compiled
    if _compiled is None:
        _compiled = _build()
    nc = _compiled

    shared = _shared_inputs()
    group_inp = [_prep_core_inputs(x, W_qkv, b_qkv, W_out, g) for g in range(2)]

    in_maps = []
    for core in range(N_CORES):
        b, g = core // 2, core % 2
        # [c4, p, ks, q]
        xT_np = (x[b].reshape(TC, 512, KSUB, 128).transpose(0, 3, 2, 1))
        m = {"xT": np.ascontiguousarray(xT_np).astype(bf16)}
        m.update(group_inp[g])
        m.update(shared)
        in_maps.append(m)

    res = run_bass_kernel_spmd(
        nc, in_maps, core_ids=list(range(N_CORES)), trace=trace,
        stitch_traces=trace,
    )
    outp = np.empty((B, T, C), dtype=np.float32)
    for b in range(B):
        outp[b] = (res.results[2 * b]["out"].astype(np.float32)
                   + res.results[2 * b + 1]["out"].astype(np.float32)
                   + b_out[None, :])
    return outp, res


def kernel(x, W_qkv, b_qkv, W_out, b_out):
    x = np.asarray(x, dtype=np.float32)
    W_qkv = np.asarray(W_qkv, dtype=np.float32)
    b_qkv = np.asarray(b_qkv, dtype=np.float32)
    W_out = np.asarray(W_out, dtype=np.float32)
    b_out = np.asarray(b_out, dtype=np.float32)
    outp, _ = run(x, W_qkv, b_qkv, W_out, b_out, trace=False)
    return outp

